# revision 2
# baseline (speedup 1.0000x reference)
"""Causal single-head attention (b=4, n=2048, d=1024, fp32) on 8 TRN2 NeuronCores.

Sharding: zig-zag q-split, core c = (batch c//2, role c%2); role0 owns
query subtiles {0,3,4,7,8,11,12,15}, role1 {1,2,5,6,9,10,13,14}. Each
core computes out rows for its own 1024 queries with all 1024 features.
The host permutes x^T columns per core (own subtiles sorted first, the
other role's last); k-chunks are processed in interleaved pair order
c = (own t, other t, ...), so chunk c covers exactly the causal q-suffix
[128*(c//2), 1024) and only the leading 128-col block of each suffix
needs a mask (triangle for even c, all-0/1 for odd c -- role-dependent
data only; the SPMD instruction stream is identical on all cores).

Pipeline per core (fp32 PSUM accumulation everywhere):
  zq[b,q] = M^T xq^T with M = W_q^T W_k host-precomputed (replaces both
  the Q and K projections); d-chunks 0:2 run as one fp8-e4m3 DoubleRow
  pass, chunks 2:8 in bf16. Scores sT[k,q] = x^T zq per k-chunk suffix
  run entirely in fp8 DoubleRow (two 128-deep k-planes per pass);
  P = exp(sT/32) * mask (scores bounded, no max subtraction); row sums
  via contiguous DVE chunk-presums + one ones-matmul per q-position;
  U^T[b,q] = x^T P with suffix-partial bf16 PSUM chains;
  out[q,o] = (U W_v^T) / l in bf16. Measured rel err ~1.6e-2 (gate 2e-2).

Schedule: two HWDGE DMA queues (sync + scalar engines) with the
zq-critical pieces first in arrival-matched order and bulk (xN, wv)
strictly after; a PE warm-up filler holds the HAM clock-gate at full
rate until real data lands; PSUM pools are scoped per stage; the last
out position is written in fine units to shorten the drain tail.
"""

import os
import sys

if os.path.isdir("/opt/trn_rl_repo") and "/opt/trn_rl_repo" not in sys.path:
    sys.path.insert(0, "/opt/trn_rl_repo")

import numpy as np
import ml_dtypes

BF16 = ml_dtypes.bfloat16

B, N, D = 4, 2048, 1024
NCORES = 8
P = 128
NKC = N // P       # 16 k chunks
NDC = D // P       # 8 d chunks
NQ = 1024          # own queries per core
NPOS = 8           # own 128-row q positions
SCALE = 1.0 / 32.0

ROLE_SUBTILES = {
    0: (0, 3, 4, 7, 8, 11, 12, 15),
    1: (1, 2, 5, 6, 9, 10, 13, 14),
}

_CACHE = {}


def _chunk_cols(c):
    """xT column block for interleaved k-chunk c (compile-time const)."""
    t = c // 2
    if c % 2 == 0:
        return t * P, (t + 1) * P          # own subtile t: cols [0, 1024)
    return NQ + t * P, NQ + (t + 1) * P    # other subtile t: cols [1024, 2048)


def _build_module():
    from concourse import bacc
    import concourse.tile as tile
    import concourse.mybir as mybir

    bf = mybir.dt.bfloat16
    f32 = mybir.dt.float32
    f8 = mybir.dt.float8e4
    DR = mybir.MatmulPerfMode.DoubleRow
    Exp = mybir.ActivationFunctionType.Exp

    nc = bacc.Bacc("TRN2", target_bir_lowering=False, debug=False, num_devices=NCORES)

    xTq_d = nc.dram_tensor("xTq", [P, NDC, NQ], bf, kind="ExternalInput")
    xq8_d = nc.dram_tensor("xq8", [P, 2, NQ], f8, kind="ExternalInput")
    m8_d = nc.dram_tensor("m8", [P, 2, D], f8, kind="ExternalInput")
    xT8_d = nc.dram_tensor("xT8", [P, NDC, N], f8, kind="ExternalInput")
    m_d = nc.dram_tensor("m", [P, NDC, D], bf, kind="ExternalInput")
    xN_d = nc.dram_tensor("xN", [P, NKC, D], bf, kind="ExternalInput")
    wv_d = nc.dram_tensor("wv", [P, NDC, D], bf, kind="ExternalInput")
    mk_d = nc.dram_tensor("masks", [P, NKC, P], bf, kind="ExternalInput")
    out_d = nc.dram_tensor("out", [NQ, D], f32, kind="ExternalOutput")

    out_r = out_d.ap().rearrange("(s p) o -> p s o", p=P)

    with tile.TileContext(nc) as tc:
        with tc.tile_pool(name="pers", bufs=1) as pers:
            xTq = pers.tile([P, NDC, NQ], bf, tag="xTq")
            xT8 = pers.tile([P, NDC, N], f8, tag="xT8")
            xN = pers.tile([P, NKC, D], bf, tag="xN")
            zq8 = pers.tile([P, NDC, NQ], f8, tag="zq8")
            wv = pers.tile([P, NDC, D], bf, tag="wv")
            sheet = pers.tile([P, NKC, NQ], bf, tag="sheet")
            uT = pers.tile([P, NDC, NQ], bf, tag="uT")
            mks = pers.tile([P, NKC, P], bf, tag="masks")
            shs = pers.tile([P, NPOS, P], f32, tag="shs")
            ones = pers.tile([P, 1], f32, tag="ones")
            wsrc = pers.tile([P, 512], bf, tag="wsrc")
            m = pers.tile([P, NDC, D], bf, tag="m")
            xq8 = pers.tile([P, 2, NQ], f8, tag="xq8")
            m8 = pers.tile([P, 2, D], f8, tag="m8")

            nc.vector.memset(ones[:], 1.0)
            nc.vector.memset(wsrc[:], 0.0)

            # Critical-path transfers split across both queues, finest first
            # so zq can start ~11us in; bulk (xN, wv) strictly after so it
            # cannot starve the critical stream of HBM bandwidth.
            nc.sync.dma_start(xq8[:], xq8_d.ap())
            nc.scalar.dma_start(m8[:], m8_d.ap())
            nc.sync.dma_start(xTq[:, 2:8, 0:256], xTq_d.ap()[:, 2:8, 0:256])
            nc.scalar.dma_start(m[:, 2:8, 0:256], m_d.ap()[:, 2:8, 0:256])
            nc.sync.dma_start(xTq[:, 2:8, 256:512], xTq_d.ap()[:, 2:8, 256:512])
            nc.scalar.dma_start(m[:, 2:8, 256:512], m_d.ap()[:, 2:8, 256:512])
            nc.sync.dma_start(xTq[:, 2:8, 512:1024], xTq_d.ap()[:, 2:8, 512:1024])
            nc.scalar.dma_start(m[:, 2:8, 512:1024], m_d.ap()[:, 2:8, 512:1024])
            nc.sync.dma_start(xT8[:], xT8_d.ap())
            nc.sync.dma_start(mks[:], mk_d.ap())
            nc.scalar.dma_start(xN[:, 0:8, :], xN_d.ap()[:, 0:8, :])
            nc.scalar.dma_start(wv[:], wv_d.ap())
            nc.scalar.dma_start(xN[:, 8:16, :], xN_d.ap()[:, 8:16, :])

            # ---- zq projection: zq[b, q] = M^T xq^T ----
            # Warm-up PSUM pool coexists with psA so zq never waits on it.
            with (
                tc.tile_pool(name="psA", bufs=4, space="PSUM") as psA,
                tc.tile_pool(name="warmps", bufs=2, space="PSUM") as warmps,
            ):
                # PE warm-up filler on garbage (keeps the HAM clock-gate busy
                # while the first DMAs land).
                for _ in range(12):
                    wps = warmps.tile([P, 512], f32, tag="warm")
                    nc.tensor.matmul(wps, wsrc[:, :P], wsrc[:], start=True, stop=True)

                # group order tracks DMA arrival: fine q-quarters over
                # bt 0..3 first, wider groups as data lands.
                zgroups = [(0, 256, range(2)), (256, 512, range(2)),
                           (0, 512, range(2, 4)),
                           (512, 1024, range(4)), (0, 512, range(4, 8)),
                           (512, 1024, range(4, 8))]
                for qa, qb, bts in zgroups:
                    for bt in bts:
                        ps = psA.tile(
                            [P, qb - qa], f32, tag="proj", name=f"z{qa}_{bt}"
                        )
                        # d-chunks 0:2 in one fp8 DoubleRow pass, rest bf16
                        nc.tensor.matmul(
                            ps,
                            m8[:, :, bt * P : (bt + 1) * P],
                            xq8[:, :, qa:qb],
                            start=True,
                            stop=False,
                            perf_mode=DR,
                        )
                        for dc in range(2, NDC):
                            nc.tensor.matmul(
                                ps,
                                m[:, dc, bt * P : (bt + 1) * P],
                                xTq[:, dc, qa:qb],
                                start=False,
                                stop=(dc == NDC - 1),
                            )
                        nc.vector.tensor_copy(zq8[:, bt, qa:qb], ps)

            # ---- attention ----
            with (
                tc.tile_pool(name="outst", bufs=4) as outst,
                tc.tile_pool(name="rcpp", bufs=8) as rcpp,
            ):
                # scores: per k-chunk suffix, fp8 DoubleRow matmuls (two
                # 128-deep k-planes per pass), exp + leading-block mask
                def presums(plo, phi):
                    # row-sum presum for positions [plo, phi): contiguous
                    # [128,128] adds over the 2p+2 contributing chunks
                    for p_ in range(plo, phi):
                        blk = slice(p_ * P, (p_ + 1) * P)
                        nc.vector.tensor_add(
                            shs[:, p_, :], sheet[:, 2 * p_, blk],
                            sheet[:, 2 * p_ + 1, blk],
                        )
                        for c in range(2 * p_):
                            nc.vector.tensor_add(
                                shs[:, p_, :], shs[:, p_, :], sheet[:, c, blk]
                            )

                with tc.tile_pool(name="stps", bufs=3, space="PSUM") as stps:
                    for c in range(NKC):
                        qs = (c // 2) * P
                        cl, ch = _chunk_cols(c)
                        seg = qs
                        while seg < NQ:
                            se = min(seg + 512, NQ)
                            ps = stps.tile(
                                [P, se - seg], f32, tag="st", name=f"st{c}_{seg}"
                            )
                            for k2 in range(NDC // 2):
                                nc.tensor.matmul(
                                    ps,
                                    xT8[:, 2 * k2 : 2 * k2 + 2, cl:ch],
                                    zq8[:, 2 * k2 : 2 * k2 + 2, seg:se],
                                    start=(k2 == 0),
                                    stop=(k2 == NDC // 2 - 1),
                                    perf_mode=DR,
                                )
                            nc.scalar.activation(
                                sheet[:, c, seg:se], ps, Exp, bias=0.0, scale=SCALE
                            )
                            if seg == qs:
                                nc.gpsimd.tensor_mul(
                                    sheet[:, c, qs : qs + P],
                                    sheet[:, c, qs : qs + P],
                                    mks[:, c, :],
                                )
                            seg = se
                        if c == 7:
                            presums(0, 4)

                attn_pools = (
                    tc.tile_pool(name="utps", bufs=3, space="PSUM"),
                    tc.tile_pool(name="outps", bufs=3, space="PSUM"),
                    tc.tile_pool(name="rsps", bufs=2, space="PSUM"),
                )

                def ut_half(qh):
                    lo, hi = qh * 512, (qh + 1) * 512
                    ncmax = 8 if qh == 0 else 16
                    for bt in range(NDC):
                        ps = utps.tile([P, 512], f32, tag="ut", name=f"ut{qh}_{bt}")
                        for c in range(ncmax):
                            qs = max((c // 2) * P, lo)
                            nc.tensor.matmul(
                                ps[:, qs - lo : 512],
                                xN[:, c, bt * P : (bt + 1) * P],
                                sheet[:, c, qs:hi],
                                start=(c == 0),
                                stop=(c == ncmax - 1),
                            )
                        nc.vector.tensor_copy(uT[:, bt, lo:hi], ps)

                def rowsums(plo, phi):
                    rs = []
                    for p_ in range(plo, phi):
                        sm = rsps.tile([P, 1], f32, tag="sm", name=f"sm{p_}")
                        nc.tensor.matmul(sm, shs[:, p_, :], ones[:], start=True, stop=True)
                        r = rcpp.tile([P, 1], f32, tag="rcp", name=f"r{p_}")
                        nc.vector.reciprocal(r[:], sm)
                        rs.append(r)
                    return rs

                def out_blocks(plo, phi, rs):
                    for p_ in range(plo, phi):
                        r = rs[p_ - plo]
                        qlo = p_ * P
                        # final position in finer units so the last
                        # norm+DMA chain off the critical path is short
                        W = 256 if p_ == NPOS - 1 else 512
                        for oh in range(D // W):
                            ps = outps.tile(
                                [P, W], f32, tag="out", name=f"o{p_}_{oh}"
                            )
                            for bc in range(NDC):
                                nc.tensor.matmul(
                                    ps,
                                    uT[:, bc, qlo : qlo + P],
                                    wv[:, bc, oh * W : (oh + 1) * W],
                                    start=(bc == 0),
                                    stop=(bc == NDC - 1),
                                )
                            ot = outst.tile([P, W], f32, tag="ot", name=f"ot{p_}_{oh}")
                            if oh % 2 == 0:
                                nc.scalar.mul(ot[:], ps, r[:])
                            else:
                                nc.vector.tensor_scalar_mul(ot[:], ps, r[:])
                            eng = nc.sync if (p_ + oh) % 2 == 0 else nc.scalar
                            eng.dma_start(
                                out_r[:, p_, oh * W : (oh + 1) * W], ot[:]
                            )

                with attn_pools[0] as utps, attn_pools[1] as outps, \
                        attn_pools[2] as rsps:
                    ut_half(0)
                    rs = rowsums(0, 4)
                    out_blocks(0, 4, rs)
                    presums(4, 8)
                    ut_half(1)
                    rs = rowsums(4, 8)
                    out_blocks(4, 8, rs)

    nc.compile()
    return nc


def _masks_np(role):
    subs = ROLE_SUBTILES[role]
    other = ROLE_SUBTILES[1 - role]
    kk = np.arange(P)[:, None]
    qq = np.arange(P)[None, :]
    tri = (kk <= qq).astype(np.float32)
    ms = np.empty((NKC, P, P), dtype=np.float32)
    for c in range(NKC):
        t = c // 2
        if c % 2 == 0:
            ms[c] = tri
        else:
            ms[c] = 1.0 if other[t] < subs[t] else 0.0
    # DRAM layout [P(k), NKC, P(q)]
    return np.ascontiguousarray(ms.transpose(1, 0, 2)).astype(BF16)


def get_module():
    if "nc" not in _CACHE:
        _CACHE["nc"] = _build_module()
    return _CACHE["nc"]


def make_in_maps(x, W_q, W_k, W_v):
    x = np.asarray(x, dtype=np.float32)
    W_q = np.asarray(W_q, dtype=np.float32)
    W_k = np.asarray(W_k, dtype=np.float32)
    m = np.ascontiguousarray(W_q.T @ W_k).astype(BF16)
    m_r = np.ascontiguousarray(m.reshape(NDC, P, D).transpose(1, 0, 2))
    m8_r = np.ascontiguousarray(m_r[:, 0:2, :]).astype(ml_dtypes.float8_e4m3)
    wvT = np.asarray(W_v, dtype=np.float32).T.astype(BF16)
    wv_r = np.ascontiguousarray(wvT.reshape(NDC, P, D).transpose(1, 0, 2))
    masks = [_masks_np(r) for r in range(2)]
    in_maps = []
    for core in range(NCORES):
        b, r = core // 2, core % 2
        perm = list(ROLE_SUBTILES[r]) + list(ROLE_SUBTILES[1 - r])
        xb = x[b].astype(BF16)                       # [n, d]
        # xT permuted: [P, NDC, N]; column order = perm subtiles
        xTp = xb.reshape(NKC, P, NDC, P)             # [st, pn, dc, pd]
        xTp = xTp[perm]                              # permuted subtiles
        xTp = np.ascontiguousarray(
            xTp.transpose(3, 2, 0, 1).reshape(P, NDC, N)
        )
        xTq = np.ascontiguousarray(xTp[:, :, 0:NQ])
        xT8 = xTp.astype(ml_dtypes.float8_e4m3)
        xq8 = np.ascontiguousarray(xT8[:, 0:2, 0:NQ])
        # xN rows in interleaved chunk order: chunk 2t = own[t], 2t+1 = other[t]
        inter = []
        for t in range(NPOS):
            inter.append(ROLE_SUBTILES[r][t])
            inter.append(ROLE_SUBTILES[1 - r][t])
        xNp = xb.reshape(NKC, P, D)[inter]           # [c, pk, d]
        xNp = np.ascontiguousarray(xNp.transpose(1, 0, 2))
        in_maps.append(
            {
                "xTq": xTq,
                "xq8": xq8,
                "m8": m8_r,
                "xT8": xT8,
                "xN": xNp,
                "m": m_r,
                "wv": wv_r,
                "masks": masks[r],
            }
        )
    return in_maps


def kernel(x, W_q, W_k, W_v):
    from concourse.bass_utils import run_bass_kernel_spmd

    nc = get_module()
    in_maps = make_in_maps(x, W_q, W_k, W_v)
    res = run_bass_kernel_spmd(
        nc,
        in_maps,
        list(range(NCORES)),
        trace=bool(int(os.environ.get("KERNEL_TRACE", "0"))),
    )
    _CACHE["last_result"] = res
    out = np.empty((B, N, D), dtype=np.float32)
    for core in range(NCORES):
        b, r = core // 2, core % 2
        res_out = res.results[core]["out"]
        for i, s in enumerate(ROLE_SUBTILES[r]):
            out[b, s * P : (s + 1) * P, :] = res_out[i * P : (i + 1) * P]
    return out
